# revision 10
# baseline (speedup 1.0000x reference)
"""Grouped-expert SwiGLU (MoE) Bass kernel for 8 TRN2 NeuronCores.

Problem: tokens pre-sorted by expert with per-expert counts; for expert e's
token slice xs: y = (silu(xs @ G_e^T) * (xs @ U_e^T)) @ D_e^T.

Strategy (all host logic; device program is uniform SPMD across 8 cores):
  * Tokens are split into 512-token blocks (counts are multiples of 512).
  * The 32 blocks are decomposed into 8 pieces of 3 blocks + 8 pieces of
    1 block, each piece single-expert; every core gets one 3-piece and one
    1-piece => exactly 2048 tokens/core, perfectly balanced compute.
  * All operands are cast to bf16 on host (PE rate identical to f32r, but
    half the HBM traffic / SBUF footprint; abs accuracy ~3e-3 rel, well
    inside the 2e-2 gate). PSUM accumulation stays f32.
  * Every DMA source is pre-packed on host into the exact [partition,
    free] contiguous layout the device needs, so DMA descriptors are
    2-16 KB/partition contiguous runs (2x packet efficiency vs the
    strided 512B gathers of the naive layout).
  * Weight streams are issued from the scalar HWDGE queue (gate/up) and
    sync HWDGE queue (down + x + y) with enough pool buffers for one-
    iteration-ahead prefetch, keeping the PE matmul stream stall-free
    and the HAM clock-gate warm.
  * Output y is written bf16 in [dim-chunk, 128, T] layout per core and
    scattered/upcast back on the host.
"""

import numpy as np
import ml_dtypes

import concourse.tile as tile
from concourse import bacc, mybir
from concourse.bass_utils import run_bass_kernel_spmd

BF16 = ml_dtypes.bfloat16
TB = 512  # token block
NCORES = 8

_PROGRAM_CACHE = {}


# --------------------------------------------------------------------------
# device program
# --------------------------------------------------------------------------
def build_program(piece_sizes, dim, hid, reps=1):
    """Uniform per-core program: for each piece i of piece_sizes[i] blocks,
    compute SwiGLU of its tokens with weight set i.

    Inputs (all bf16, host-prepacked so each DMA is contiguous/partition):
      xb        [nblk, 128, KD*TB]   x block b: [p][kd][t] = x[b*TB+t, kd*128+p]
      g{i},u{i} [HB, 128, KD*128]    [hb][p][kd][h] = W[hb*128+h, kd*128+p]
      d{i}      [NCH, 128, HB*128]   [cb][p][hb][c] = D[cb*128+c, hb*128+p]
    Output:
      y         [NCH, 128, T]  bf16  [cb][p][t] = out[t, cb*128+p]
    """
    key = (tuple(piece_sizes), dim, hid, reps)
    if key in _PROGRAM_CACHE:
        return _PROGRAM_CACHE[key]

    f32 = mybir.dt.float32
    bf16 = mybir.dt.bfloat16
    KD = dim // 128   # k-tiles for gate/up contraction
    HB = hid // 128   # h-tiles
    NCH = dim // 128  # output dim chunks
    nblk = sum(piece_sizes)
    T = nblk * TB

    nc = bacc.Bacc("TRN2", target_bir_lowering=False, debug=False, num_devices=NCORES)
    xb = nc.dram_tensor("xb", [nblk, 128, KD * TB], bf16, kind="ExternalInput").ap()
    gs, us, ds = [], [], []
    for i in range(len(piece_sizes)):
        gs.append(nc.dram_tensor(f"g{i}", [HB, 128, KD * 128], bf16, kind="ExternalInput").ap())
        us.append(nc.dram_tensor(f"u{i}", [HB, 128, KD * 128], bf16, kind="ExternalInput").ap())
        ds.append(nc.dram_tensor(f"d{i}", [NCH, 128, HB * 128], bf16, kind="ExternalInput").ap())
    y = nc.dram_tensor("y", [NCH, 128, T], bf16, kind="ExternalOutput").ap()

    max_sz = max(piece_sizes)
    with tile.TileContext(nc) as tc:
        with (
            tc.tile_pool(name="xp", bufs=min(nblk, max_sz + 1)) as xp,
            tc.tile_pool(name="h1p", bufs=min(2, len(piece_sizes))) as h1p,
            tc.tile_pool(name="wp", bufs=2) as wp,
            tc.tile_pool(name="dwp", bufs=6) as dwp,
            tc.tile_pool(name="actp", bufs=3) as actp,
            tc.tile_pool(name="otp", bufs=3) as otp,
            # pool slots are per-tag: psgu holds psg+psu tags (2 bufs each =
            # 4 banks), psop 3 banks -> 7 of 8 PSUM banks
            tc.tile_pool(name="psgu", bufs=2, space="PSUM") as psgu,
            tc.tile_pool(name="psop", bufs=3, space="PSUM") as psop,
        ):
          for _rep in range(reps):
            blk0 = 0  # global block index
            for pi, sz in enumerate(piece_sizes):
                Tp = sz * TB
                h1 = h1p.tile([128, HB, Tp], bf16, tag="h1")
                # ---- phase 1: h1[h, t] = silu(G^T x) * (U^T x).
                # x blocks are separate tiles; each block's DMA is split in
                # two halves so the first matmuls start after ~half a block
                # has landed. gate/up weights stream once per piece
                # (hb-outer loop) from the scalar HWDGE queue.
                xws = []
                for tb in range(sz):
                    xw = xp.tile([128, KD, TB], bf16, tag="x")
                    q = KD // 4  # quarter-split so the first matmuls start
                    for j in range(4):  # after ~512KB instead of ~2MB
                        nc.sync.dma_start(
                            out=xw[:, j * q : (j + 1) * q, :],
                            in_=xb[
                                blk0 + tb, :, j * q * TB : (j + 1) * q * TB
                            ].rearrange("p (kd t) -> p kd t", t=TB),
                        )
                    xws.append(xw)
                PF = min(6, NCH)
                dws = {}
                for hb in range(HB):
                    gw = wp.tile([128, KD, 128], bf16, tag="gw")
                    uw = wp.tile([128, KD, 128], bf16, tag="uw")
                    half = KD // 2
                    for w, src in ((gw, gs[pi]), (uw, us[pi])):
                        for j in range(2):
                            nc.scalar.dma_start(
                                out=w[:, j * half : (j + 1) * half, :],
                                in_=src[
                                    hb, :, j * half * 128 : (j + 1) * half * 128
                                ].rearrange("p (kd h) -> p kd h", h=128),
                            )
                    if hb == 0:
                        # hoisted down-proj prefetch: the first PF dw tiles
                        # start loading right behind hb=0's gate/up weights,
                        # so phase 2 begins with a deep resident pipeline
                        for dcb in range(PF):
                            dw = dwp.tile([128, HB, 128], bf16, tag="dw")
                            nc.scalar.dma_start(
                                out=dw,
                                in_=ds[pi][dcb].rearrange(
                                    "p (hb c) -> p hb c", c=128
                                ),
                            )
                            dws[dcb] = dw
                    for tb in range(sz):
                        xw = xws[tb]
                        psg = psgu.tile([128, TB], f32, tag="psg")
                        psu = psgu.tile([128, TB], f32, tag="psu")
                        for kd in range(KD):
                            nc.tensor.matmul(
                                psg,
                                gw[:, kd, :],
                                xw[:, kd, :],
                                start=(kd == 0),
                                stop=(kd == KD - 1),
                            )
                        for kd in range(KD):
                            nc.tensor.matmul(
                                psu,
                                uw[:, kd, :],
                                xw[:, kd, :],
                                start=(kd == 0),
                                stop=(kd == KD - 1),
                            )
                        act = actp.tile([128, TB], f32, tag="act")
                        nc.scalar.activation(
                            act, psg, mybir.ActivationFunctionType.Silu
                        )
                        nc.vector.tensor_mul(
                            h1[:, hb, tb * TB : (tb + 1) * TB], act, psu
                        )
                # ---- phase 2: yT[c, t] = sum_h D^T[h, c] * h1[h, t]
                # stationary = 128x128 D^T column tiles, moving = h1
                # (already resident). dw prefetch rolls PF iterations ahead
                # on the scalar queue (idle during phase 2).
                for dcb in range(NCH):
                    if dcb + PF < NCH:
                        dw = dwp.tile([128, HB, 128], bf16, tag="dw")
                        nc.scalar.dma_start(
                            out=dw,
                            in_=ds[pi][dcb + PF].rearrange(
                                "p (hb c) -> p hb c", c=128
                            ),
                        )
                        dws[dcb + PF] = dw
                    dw = dws.pop(dcb)
                    for tcol in range(Tp // 512):
                        pso = psop.tile([128, 512], f32, tag="pso")
                        for hb in range(HB):
                            nc.tensor.matmul(
                                pso,
                                dw[:, hb, :],
                                h1[:, hb, tcol * 512 : (tcol + 1) * 512],
                                start=(hb == 0),
                                stop=(hb == HB - 1),
                            )
                        ot = otp.tile([128, 512], bf16, tag="ot")
                        nc.vector.tensor_copy(ot, pso)
                        nc.sync.dma_start(
                            out=y[
                                dcb,
                                :,
                                blk0 * TB + tcol * 512 : blk0 * TB + (tcol + 1) * 512,
                            ],
                            in_=ot,
                        )
                blk0 += sz
    nc.move_matmul_waits_to_ldweights()
    nc.compile()
    _PROGRAM_CACHE[key] = nc
    return nc


# --------------------------------------------------------------------------
# host-side planning
# --------------------------------------------------------------------------
def plan_pieces(block_counts):
    """Decompose per-expert block counts into 8 cores x uniform piece sizes.

    Returns (piece_sizes, plans) where plans[core] = [(expert, block_start,
    nblocks), ...] with block_start in global padded block coordinates.
    Tries the (3, 1) split (balanced, min weight traffic); falls back to
    single-block pieces.
    """
    E = len(block_counts)
    starts = np.zeros(E, dtype=np.int64)
    np.cumsum(block_counts[:-1], out=starts[1:])
    total = int(np.sum(block_counts))

    if total == 4 * NCORES:
        # try k3[e] three-pieces + k1[e] singles with sum(k3) == 8
        k3 = [int(c) // 3 for c in block_counts]
        while sum(k3) > NCORES:
            e = max(range(E), key=lambda i: k3[i])
            k3[e] -= 1
        if sum(k3) == NCORES:
            threes, ones = [], []
            for e in range(E):
                b = int(block_counts[e])
                s = int(starts[e])
                for _ in range(k3[e]):
                    threes.append((e, s, 3))
                    s += 3
                while s < int(starts[e]) + b:
                    ones.append((e, s, 1))
                    s += 1
            assert len(threes) == NCORES and len(ones) == NCORES
            # pair same-expert pieces on the same core where possible;
            # SMALL piece first: its first matmuls need only ~2MB of x, so
            # the PE starts (and warms the clock) while the big piece's
            # x blocks prefetch; the big piece's robust phase 2 lands at
            # the tail, keeping the clock warm to the end
            plans = []
            used1 = [False] * NCORES
            for t in threes:
                j = next(
                    (
                        i
                        for i in range(NCORES)
                        if not used1[i] and ones[i][0] == t[0]
                    ),
                    None,
                )
                if j is None:
                    j = next(i for i in range(NCORES) if not used1[i])
                used1[j] = True
                plans.append([ones[j], t])
            return (1, 3), plans

    # fallback: single-block pieces, padded to a multiple of NCORES with
    # dummy zero blocks (expert 0 weights, output discarded)
    per_core = -(-total // NCORES)
    pieces = []
    for e in range(E):
        for b in range(int(block_counts[e])):
            pieces.append((e, int(starts[e]) + b, 1))
    while len(pieces) < per_core * NCORES:
        pieces.append((0, -1, 1))  # dummy
    plans = [pieces[c * per_core : (c + 1) * per_core] for c in range(NCORES)]
    return tuple([1] * per_core), plans


def _pack_gu(w, KD, HB):
    """[hid, dim] f32 -> [HB, 128, KD*128] bf16 with
    out[hb, p, kd*128+h] = w[hb*128+h, kd*128+p]."""
    hid, dim = w.shape
    arr = w.reshape(HB, 128, KD, 128).transpose(0, 3, 2, 1)  # hb, p, kd, h
    return np.ascontiguousarray(arr.astype(BF16).reshape(HB, 128, KD * 128))


def _pack_d(w, NCH, HB):
    """[dim, hid] f32 -> [NCH, 128, HB*128] bf16 with
    out[cb, p, hb*128+c] = w[cb*128+c, hb*128+p]."""
    dim, hid = w.shape
    arr = w.reshape(NCH, 128, HB, 128).transpose(0, 3, 2, 1)  # cb, p, hb, c
    return np.ascontiguousarray(arr.astype(BF16).reshape(NCH, 128, HB * 128))


def _pack_x(xc, KD):
    """[T, dim] f32 -> [nblk, 128, KD*TB] bf16 with
    out[b, p, kd*TB+t] = x[b*TB+t, kd*128+p]."""
    T, dim = xc.shape
    nblk = T // TB
    arr = xc.reshape(nblk, TB, KD, 128).transpose(0, 3, 2, 1)  # b, p, kd, t
    return np.ascontiguousarray(arr.astype(BF16).reshape(nblk, 128, KD * TB))


def prepare(x, gate_proj, up_proj, down_proj, num_tokens_per_expert):
    """Host-side planning + per-core input construction.

    Returns (piece_sizes, plans, in_maps, scatter_info).
    """
    x = np.ascontiguousarray(np.asarray(x, dtype=np.float32))
    gate_proj = np.asarray(gate_proj, dtype=np.float32)
    up_proj = np.asarray(up_proj, dtype=np.float32)
    down_proj = np.asarray(down_proj, dtype=np.float32)
    counts = np.asarray(num_tokens_per_expert).astype(np.int64)

    T, dim = x.shape
    E, hid, _ = gate_proj.shape
    KD, HB, NCH = dim // 128, hid // 128, dim // 128

    # ---- pad each expert's token segment to a multiple of TB (no-op for the
    # staged problem where every count is already a multiple of 512)
    offs = np.concatenate([[0], np.cumsum(counts)])
    pad_counts = ((counts + TB - 1) // TB) * TB
    if np.array_equal(pad_counts, counts):
        x_pad = x
        pad_offs = offs
        padded = False
    else:
        pad_offs = np.concatenate([[0], np.cumsum(pad_counts)])
        x_pad = np.zeros((int(pad_offs[-1]), dim), dtype=np.float32)
        for e in range(E):
            x_pad[pad_offs[e] : pad_offs[e] + counts[e]] = x[offs[e] : offs[e + 1]]
        padded = True

    block_counts = pad_counts // TB
    piece_sizes, plans = plan_pieces(block_counts)

    # ---- per-expert packed bf16 weights (shared across cores)
    GP = [_pack_gu(gate_proj[e], KD, HB) for e in range(E)]
    UP = [_pack_gu(up_proj[e], KD, HB) for e in range(E)]
    DP = [_pack_d(down_proj[e], NCH, HB) for e in range(E)]

    in_maps = []
    for c in range(NCORES):
        plan = plans[c]
        xs = []
        for (e, bs, nb) in plan:
            if bs < 0:
                xs.append(np.zeros((nb * TB, dim), dtype=np.float32))
            else:
                xs.append(x_pad[bs * TB : (bs + nb) * TB])
        xc = np.concatenate(xs, axis=0) if len(xs) > 1 else xs[0]
        m = {"xb": _pack_x(xc, KD)}
        for i, (e, bs, nb) in enumerate(plan):
            m[f"g{i}"] = GP[e]
            m[f"u{i}"] = UP[e]
            m[f"d{i}"] = DP[e]
        in_maps.append(m)

    scatter_info = (T, dim, E, offs, pad_offs, counts, padded)
    return piece_sizes, plans, in_maps, scatter_info


def scatter(per_core_y, plans, scatter_info):
    """Assemble the full output from per-core y arrays."""
    T, dim, E, offs, pad_offs, counts, padded = scatter_info
    out_pad = np.empty((int(pad_offs[-1]), dim), dtype=np.float32)
    for c in range(NCORES):
        yc = np.asarray(per_core_y[c]).astype(np.float32)  # [NCH, 128, T_core]
        yc = yc.transpose(2, 0, 1).reshape(yc.shape[2], dim)  # [T_core, dim]
        t = 0
        for (e, bs, nb) in plans[c]:
            if bs >= 0:
                out_pad[bs * TB : (bs + nb) * TB] = yc[t : t + nb * TB]
            t += nb * TB
    if not padded:
        return out_pad
    out = np.empty((T, dim), dtype=np.float32)
    for e in range(E):
        out[offs[e] : offs[e + 1]] = out_pad[pad_offs[e] : pad_offs[e] + counts[e]]
    return out


def kernel(x, gate_proj, up_proj, down_proj, num_tokens_per_expert):
    piece_sizes, plans, in_maps, scatter_info = prepare(
        x, gate_proj, up_proj, down_proj, num_tokens_per_expert
    )
    dim = scatter_info[1]
    hid = np.asarray(gate_proj).shape[1]
    nc = build_program(piece_sizes, dim, hid)
    res = run_bass_kernel_spmd(nc, in_maps, core_ids=list(range(NCORES)))
    return scatter([res.results[c]["y"] for c in range(NCORES)], plans, scatter_info)


# revision 13
# speedup vs baseline: 1.0093x; 1.0093x over previous
"""Grouped-expert SwiGLU (MoE) Bass kernel for 8 TRN2 NeuronCores.

Problem: tokens pre-sorted by expert with per-expert counts; for expert e's
token slice xs: y = (silu(xs @ G_e^T) * (xs @ U_e^T)) @ D_e^T.

Strategy (all host logic; device program is uniform SPMD across 8 cores):
  * Tokens are split into 512-token blocks (counts are multiples of 512).
  * The 32 blocks are decomposed into 8 pieces of 3 blocks + 8 pieces of
    1 block, each piece single-expert; every core gets one 3-piece and one
    1-piece => exactly 2048 tokens/core, perfectly balanced compute.
  * All operands are cast to bf16 on host (PE rate identical to f32r, but
    half the HBM traffic / SBUF footprint; abs accuracy ~3e-3 rel, well
    inside the 2e-2 gate). PSUM accumulation stays f32.
  * Every DMA source is pre-packed on host into the exact [partition,
    free] contiguous layout the device needs, so DMA descriptors are
    2-16 KB/partition contiguous runs (2x packet efficiency vs the
    strided 512B gathers of the naive layout).
  * Weight streams are issued from the scalar HWDGE queue (gate/up) and
    sync HWDGE queue (down + x + y) with enough pool buffers for one-
    iteration-ahead prefetch, keeping the PE matmul stream stall-free
    and the HAM clock-gate warm.
  * Output y is written bf16 in [dim-chunk, 128, T] layout per core and
    scattered/upcast back on the host.
"""

import numpy as np
import ml_dtypes

import concourse.tile as tile
from concourse import bacc, mybir
from concourse.bass_utils import run_bass_kernel_spmd

BF16 = ml_dtypes.bfloat16
TB = 512  # token block
NCORES = 8

_PROGRAM_CACHE = {}


# --------------------------------------------------------------------------
# device program
# --------------------------------------------------------------------------
def build_program(piece_sizes, dim, hid, reps=1):
    """Uniform per-core program: for each piece i of piece_sizes[i] blocks,
    compute SwiGLU of its tokens with weight set i.

    Inputs (all bf16, host-prepacked so each DMA is contiguous/partition):
      xb        [nblk, 128, KD*TB]   x block b: [p][kd][t] = x[b*TB+t, kd*128+p]
      g{i},u{i} [HB, 128, KD*128]    [hb][p][kd][h] = W[hb*128+h, kd*128+p]
      d{i}      [NCH, 128, HB*128]   [cb][p][hb][c] = D[cb*128+c, hb*128+p]
    Output:
      y         [NCH, 128, T]  bf16  [cb][p][t] = out[t, cb*128+p]
    """
    key = (tuple(piece_sizes), dim, hid, reps)
    if key in _PROGRAM_CACHE:
        return _PROGRAM_CACHE[key]

    f32 = mybir.dt.float32
    bf16 = mybir.dt.bfloat16
    KD = dim // 128   # k-tiles for gate/up contraction
    HB = hid // 128   # h-tiles
    NCH = dim // 128  # output dim chunks
    nblk = sum(piece_sizes)
    T = nblk * TB

    nc = bacc.Bacc("TRN2", target_bir_lowering=False, debug=False, num_devices=NCORES)
    xb = nc.dram_tensor("xb", [nblk, 128, KD * TB], bf16, kind="ExternalInput").ap()
    gs, us, ds = [], [], []
    for i in range(len(piece_sizes)):
        gs.append(nc.dram_tensor(f"g{i}", [HB, 128, KD * 128], bf16, kind="ExternalInput").ap())
        us.append(nc.dram_tensor(f"u{i}", [HB, 128, KD * 128], bf16, kind="ExternalInput").ap())
        ds.append(nc.dram_tensor(f"d{i}", [NCH, 128, HB * 128], bf16, kind="ExternalInput").ap())
    y = nc.dram_tensor("y", [NCH, 128, T], bf16, kind="ExternalOutput").ap()

    max_sz = max(piece_sizes)
    with tile.TileContext(nc) as tc:
        with (
            tc.tile_pool(name="xp", bufs=max_sz) as xp,
            tc.tile_pool(name="h1p", bufs=min(2, len(piece_sizes))) as h1p,
            tc.tile_pool(name="wp", bufs=2) as wp,
            tc.tile_pool(name="dwp", bufs=6) as dwp,
            tc.tile_pool(name="actp", bufs=3) as actp,
            tc.tile_pool(name="otp", bufs=3) as otp,
            # pool slots are per-tag: psgu holds psg+psu tags (2 bufs each =
            # 4 banks), psop 3 banks -> 7 of 8 PSUM banks
            tc.tile_pool(name="psgu", bufs=2, space="PSUM") as psgu,
            tc.tile_pool(name="psop", bufs=3, space="PSUM") as psop,
        ):
          for _rep in range(reps):
            blk0 = 0  # global block index
            for pi, sz in enumerate(piece_sizes):
                Tp = sz * TB
                h1 = h1p.tile([128, HB, Tp], bf16, tag="h1")
                # ---- phase 1: h1[h, t] = silu(G^T x) * (U^T x).
                # x blocks are separate tiles; each block's DMA is split in
                # two halves so the first matmuls start after ~half a block
                # has landed. gate/up weights stream once per piece
                # (hb-outer loop) from the scalar HWDGE queue.
                xws = []
                for tb in range(sz):
                    xw = xp.tile([128, KD, TB], bf16, tag="x")
                    q = KD // 4  # quarter-split so the first matmuls start
                    for j in range(4):  # after ~512KB instead of ~2MB
                        nc.sync.dma_start(
                            out=xw[:, j * q : (j + 1) * q, :],
                            in_=xb[
                                blk0 + tb, :, j * q * TB : (j + 1) * q * TB
                            ].rearrange("p (kd t) -> p kd t", t=TB),
                        )
                    xws.append(xw)
                PF = min(6, NCH)
                dws = {}
                for hb in range(HB):
                    gw = wp.tile([128, KD, 128], bf16, tag="gw")
                    uw = wp.tile([128, KD, 128], bf16, tag="uw")
                    half = KD // 2
                    for w, src in ((gw, gs[pi]), (uw, us[pi])):
                        for j in range(2):
                            nc.scalar.dma_start(
                                out=w[:, j * half : (j + 1) * half, :],
                                in_=src[
                                    hb, :, j * half * 128 : (j + 1) * half * 128
                                ].rearrange("p (kd h) -> p kd h", h=128),
                            )
                    # hoisted down-proj prefetch: the first PF dw tiles load
                    # before phase 2 so it begins with a deep resident
                    # pipeline. For the first piece defer to the last hb --
                    # during startup every HBM byte is needed for x/gate/up.
                    if hb == (0 if pi else HB - 1):
                        for dcb in range(PF):
                            dw = dwp.tile([128, HB, 128], bf16, tag="dw")
                            nc.scalar.dma_start(
                                out=dw,
                                in_=ds[pi][dcb].rearrange(
                                    "p (hb c) -> p hb c", c=128
                                ),
                            )
                            dws[dcb] = dw
                    for tb in range(sz):
                        xw = xws[tb]
                        psg = psgu.tile([128, TB], f32, tag="psg")
                        psu = psgu.tile([128, TB], f32, tag="psu")
                        for kd in range(KD):
                            nc.tensor.matmul(
                                psg,
                                gw[:, kd, :],
                                xw[:, kd, :],
                                start=(kd == 0),
                                stop=(kd == KD - 1),
                            )
                        for kd in range(KD):
                            nc.tensor.matmul(
                                psu,
                                uw[:, kd, :],
                                xw[:, kd, :],
                                start=(kd == 0),
                                stop=(kd == KD - 1),
                            )
                        act = actp.tile([128, TB], f32, tag="act")
                        nc.scalar.activation(
                            act, psg, mybir.ActivationFunctionType.Silu
                        )
                        nc.vector.tensor_mul(
                            h1[:, hb, tb * TB : (tb + 1) * TB], act, psu
                        )
                # ---- phase 2: yT[c, t] = sum_h D^T[h, c] * h1[h, t]
                # stationary = 128x128 D^T column tiles, moving = h1
                # (already resident). dw prefetch rolls PF iterations ahead
                # on the scalar queue (idle during phase 2).
                for dcb in range(NCH):
                    if dcb + PF < NCH:
                        dw = dwp.tile([128, HB, 128], bf16, tag="dw")
                        nc.scalar.dma_start(
                            out=dw,
                            in_=ds[pi][dcb + PF].rearrange(
                                "p (hb c) -> p hb c", c=128
                            ),
                        )
                        dws[dcb + PF] = dw
                    dw = dws.pop(dcb)
                    for tcol in range(Tp // 512):
                        pso = psop.tile([128, 512], f32, tag="pso")
                        for hb in range(HB):
                            nc.tensor.matmul(
                                pso,
                                dw[:, hb, :],
                                h1[:, hb, tcol * 512 : (tcol + 1) * 512],
                                start=(hb == 0),
                                stop=(hb == HB - 1),
                            )
                        ot = otp.tile([128, 512], bf16, tag="ot")
                        nc.vector.tensor_copy(ot, pso)
                        nc.sync.dma_start(
                            out=y[
                                dcb,
                                :,
                                blk0 * TB + tcol * 512 : blk0 * TB + (tcol + 1) * 512,
                            ],
                            in_=ot,
                        )
                blk0 += sz
    nc.move_matmul_waits_to_ldweights()
    nc.compile()
    _PROGRAM_CACHE[key] = nc
    return nc


# --------------------------------------------------------------------------
# host-side planning
# --------------------------------------------------------------------------
def plan_pieces(block_counts):
    """Decompose per-expert block counts into 8 cores x uniform piece sizes.

    Returns (piece_sizes, plans) where plans[core] = [(expert, block_start,
    nblocks), ...] with block_start in global padded block coordinates.
    Tries the (3, 1) split (balanced, min weight traffic); falls back to
    single-block pieces.
    """
    E = len(block_counts)
    starts = np.zeros(E, dtype=np.int64)
    np.cumsum(block_counts[:-1], out=starts[1:])
    total = int(np.sum(block_counts))

    if total == 4 * NCORES:
        # try k3[e] three-pieces + k1[e] singles with sum(k3) == 8
        k3 = [int(c) // 3 for c in block_counts]
        while sum(k3) > NCORES:
            e = max(range(E), key=lambda i: k3[i])
            k3[e] -= 1
        if sum(k3) == NCORES:
            threes, ones = [], []
            for e in range(E):
                b = int(block_counts[e])
                s = int(starts[e])
                for _ in range(k3[e]):
                    threes.append((e, s, 3))
                    s += 3
                while s < int(starts[e]) + b:
                    ones.append((e, s, 1))
                    s += 1
            assert len(threes) == NCORES and len(ones) == NCORES
            # pair same-expert pieces on the same core where possible;
            # SMALL piece first: its first matmuls need only ~2MB of x, so
            # the PE starts (and warms the clock) while the big piece's
            # x blocks prefetch; the big piece's robust phase 2 lands at
            # the tail, keeping the clock warm to the end
            plans = []
            used1 = [False] * NCORES
            for t in threes:
                j = next(
                    (
                        i
                        for i in range(NCORES)
                        if not used1[i] and ones[i][0] == t[0]
                    ),
                    None,
                )
                if j is None:
                    j = next(i for i in range(NCORES) if not used1[i])
                used1[j] = True
                plans.append([t, ones[j]])
            return (3, 1), plans

    # fallback: single-block pieces, padded to a multiple of NCORES with
    # dummy zero blocks (expert 0 weights, output discarded)
    per_core = -(-total // NCORES)
    pieces = []
    for e in range(E):
        for b in range(int(block_counts[e])):
            pieces.append((e, int(starts[e]) + b, 1))
    while len(pieces) < per_core * NCORES:
        pieces.append((0, -1, 1))  # dummy
    plans = [pieces[c * per_core : (c + 1) * per_core] for c in range(NCORES)]
    return tuple([1] * per_core), plans


def _pack_gu(w, KD, HB):
    """[hid, dim] f32 -> [HB, 128, KD*128] bf16 with
    out[hb, p, kd*128+h] = w[hb*128+h, kd*128+p]."""
    hid, dim = w.shape
    arr = w.reshape(HB, 128, KD, 128).transpose(0, 3, 2, 1)  # hb, p, kd, h
    return np.ascontiguousarray(arr.astype(BF16).reshape(HB, 128, KD * 128))


def _pack_d(w, NCH, HB):
    """[dim, hid] f32 -> [NCH, 128, HB*128] bf16 with
    out[cb, p, hb*128+c] = w[cb*128+c, hb*128+p]."""
    dim, hid = w.shape
    arr = w.reshape(NCH, 128, HB, 128).transpose(0, 3, 2, 1)  # cb, p, hb, c
    return np.ascontiguousarray(arr.astype(BF16).reshape(NCH, 128, HB * 128))


def _pack_x(xc, KD):
    """[T, dim] f32 -> [nblk, 128, KD*TB] bf16 with
    out[b, p, kd*TB+t] = x[b*TB+t, kd*128+p]."""
    T, dim = xc.shape
    nblk = T // TB
    arr = xc.reshape(nblk, TB, KD, 128).transpose(0, 3, 2, 1)  # b, p, kd, t
    return np.ascontiguousarray(arr.astype(BF16).reshape(nblk, 128, KD * TB))


def prepare(x, gate_proj, up_proj, down_proj, num_tokens_per_expert):
    """Host-side planning + per-core input construction.

    Returns (piece_sizes, plans, in_maps, scatter_info).
    """
    x = np.ascontiguousarray(np.asarray(x, dtype=np.float32))
    gate_proj = np.asarray(gate_proj, dtype=np.float32)
    up_proj = np.asarray(up_proj, dtype=np.float32)
    down_proj = np.asarray(down_proj, dtype=np.float32)
    counts = np.asarray(num_tokens_per_expert).astype(np.int64)

    T, dim = x.shape
    E, hid, _ = gate_proj.shape
    KD, HB, NCH = dim // 128, hid // 128, dim // 128

    # ---- pad each expert's token segment to a multiple of TB (no-op for the
    # staged problem where every count is already a multiple of 512)
    offs = np.concatenate([[0], np.cumsum(counts)])
    pad_counts = ((counts + TB - 1) // TB) * TB
    if np.array_equal(pad_counts, counts):
        x_pad = x
        pad_offs = offs
        padded = False
    else:
        pad_offs = np.concatenate([[0], np.cumsum(pad_counts)])
        x_pad = np.zeros((int(pad_offs[-1]), dim), dtype=np.float32)
        for e in range(E):
            x_pad[pad_offs[e] : pad_offs[e] + counts[e]] = x[offs[e] : offs[e + 1]]
        padded = True

    block_counts = pad_counts // TB
    piece_sizes, plans = plan_pieces(block_counts)

    # ---- per-expert packed bf16 weights (shared across cores)
    GP = [_pack_gu(gate_proj[e], KD, HB) for e in range(E)]
    UP = [_pack_gu(up_proj[e], KD, HB) for e in range(E)]
    DP = [_pack_d(down_proj[e], NCH, HB) for e in range(E)]

    in_maps = []
    for c in range(NCORES):
        plan = plans[c]
        xs = []
        for (e, bs, nb) in plan:
            if bs < 0:
                xs.append(np.zeros((nb * TB, dim), dtype=np.float32))
            else:
                xs.append(x_pad[bs * TB : (bs + nb) * TB])
        xc = np.concatenate(xs, axis=0) if len(xs) > 1 else xs[0]
        m = {"xb": _pack_x(xc, KD)}
        for i, (e, bs, nb) in enumerate(plan):
            m[f"g{i}"] = GP[e]
            m[f"u{i}"] = UP[e]
            m[f"d{i}"] = DP[e]
        in_maps.append(m)

    scatter_info = (T, dim, E, offs, pad_offs, counts, padded)
    return piece_sizes, plans, in_maps, scatter_info


def scatter(per_core_y, plans, scatter_info):
    """Assemble the full output from per-core y arrays."""
    T, dim, E, offs, pad_offs, counts, padded = scatter_info
    out_pad = np.empty((int(pad_offs[-1]), dim), dtype=np.float32)
    for c in range(NCORES):
        yc = np.asarray(per_core_y[c]).astype(np.float32)  # [NCH, 128, T_core]
        yc = yc.transpose(2, 0, 1).reshape(yc.shape[2], dim)  # [T_core, dim]
        t = 0
        for (e, bs, nb) in plans[c]:
            if bs >= 0:
                out_pad[bs * TB : (bs + nb) * TB] = yc[t : t + nb * TB]
            t += nb * TB
    if not padded:
        return out_pad
    out = np.empty((T, dim), dtype=np.float32)
    for e in range(E):
        out[offs[e] : offs[e + 1]] = out_pad[pad_offs[e] : pad_offs[e] + counts[e]]
    return out


def kernel(x, gate_proj, up_proj, down_proj, num_tokens_per_expert):
    piece_sizes, plans, in_maps, scatter_info = prepare(
        x, gate_proj, up_proj, down_proj, num_tokens_per_expert
    )
    dim = scatter_info[1]
    hid = np.asarray(gate_proj).shape[1]
    nc = build_program(piece_sizes, dim, hid)
    res = run_bass_kernel_spmd(nc, in_maps, core_ids=list(range(NCORES)))
    return scatter([res.results[c]["y"] for c in range(NCORES)], plans, scatter_info)


# revision 16
# speedup vs baseline: 1.0128x; 1.0035x over previous
"""Grouped-expert SwiGLU (MoE) Bass kernel for 8 TRN2 NeuronCores.

Problem: tokens pre-sorted by expert with per-expert counts; for expert e's
token slice xs: y = (silu(xs @ G_e^T) * (xs @ U_e^T)) @ D_e^T.

Strategy (all host logic; device program is uniform SPMD across 8 cores):
  * Tokens are split into 512-token blocks (counts are multiples of 512).
  * The 32 blocks are decomposed into 8 pieces of 3 blocks + 8 pieces of
    1 block, each piece single-expert; every core gets one 3-piece and one
    1-piece => exactly 2048 tokens/core, perfectly balanced compute.
  * All operands are cast to bf16 on host (PE rate identical to f32r, but
    half the HBM traffic / SBUF footprint; abs accuracy ~3e-3 rel, well
    inside the 2e-2 gate). PSUM accumulation stays f32.
  * Every DMA source is pre-packed on host into the exact [partition,
    free] contiguous layout the device needs, so DMA descriptors are
    2-16 KB/partition contiguous runs (2x packet efficiency vs the
    strided 512B gathers of the naive layout).
  * Weight streams are issued from the scalar HWDGE queue (gate/up) and
    sync HWDGE queue (down + x + y) with enough pool buffers for one-
    iteration-ahead prefetch, keeping the PE matmul stream stall-free
    and the HAM clock-gate warm.
  * Output y is written bf16 in [dim-chunk, 128, T] layout per core and
    scattered/upcast back on the host.
"""

import numpy as np
import ml_dtypes

import concourse.tile as tile
from concourse import bacc, mybir
from concourse.bass_utils import run_bass_kernel_spmd

BF16 = ml_dtypes.bfloat16
TB = 512  # token block
NCORES = 8

_PROGRAM_CACHE = {}


# --------------------------------------------------------------------------
# device program
# --------------------------------------------------------------------------
def build_program(piece_sizes, dim, hid, reps=1):
    """Uniform per-core program: for each piece i of piece_sizes[i] blocks,
    compute SwiGLU of its tokens with weight set i.

    Inputs (all bf16, host-prepacked so each DMA is contiguous/partition):
      xb        [nblk, 128, KD*TB]   x block b: [p][kd][t] = x[b*TB+t, kd*128+p]
      g{i},u{i} [HB, 128, KD*128]    [hb][p][kd][h] = W[hb*128+h, kd*128+p]
      d{i}      [NCH, 128, HB*128]   [cb][p][hb][c] = D[cb*128+c, hb*128+p]
    Output:
      y         [NCH, 128, T]  bf16  [cb][p][t] = out[t, cb*128+p]
    """
    key = (tuple(piece_sizes), dim, hid, reps)
    if key in _PROGRAM_CACHE:
        return _PROGRAM_CACHE[key]

    f32 = mybir.dt.float32
    bf16 = mybir.dt.bfloat16
    KD = dim // 128   # k-tiles for gate/up contraction
    HB = hid // 128   # h-tiles
    NCH = dim // 128  # output dim chunks
    nblk = sum(piece_sizes)
    T = nblk * TB

    nc = bacc.Bacc("TRN2", target_bir_lowering=False, debug=False, num_devices=NCORES)
    xb = nc.dram_tensor("xb", [nblk, 128, KD * TB], bf16, kind="ExternalInput").ap()
    gs, us, ds = [], [], []
    for i in range(len(piece_sizes)):
        gs.append(nc.dram_tensor(f"g{i}", [HB, 128, KD * 128], bf16, kind="ExternalInput").ap())
        us.append(nc.dram_tensor(f"u{i}", [HB, 128, KD * 128], bf16, kind="ExternalInput").ap())
        ds.append(nc.dram_tensor(f"d{i}", [NCH, 128, HB * 128], bf16, kind="ExternalInput").ap())
    y = nc.dram_tensor("y", [NCH, 128, T], bf16, kind="ExternalOutput").ap()

    max_sz = max(piece_sizes)
    with tile.TileContext(nc) as tc:
        with (
            tc.tile_pool(name="xp", bufs=max_sz) as xp,
            tc.tile_pool(name="h1p", bufs=min(2, len(piece_sizes))) as h1p,
            tc.tile_pool(name="wp", bufs=2) as wp,
            tc.tile_pool(name="dwp", bufs=5) as dwp,
            tc.tile_pool(name="actp", bufs=3) as actp,
            tc.tile_pool(name="otp", bufs=3) as otp,
            # pool slots are per-tag: psgu holds psg+psu tags (2 bufs each =
            # 4 banks), psop 3 banks -> 7 of 8 PSUM banks
            tc.tile_pool(name="psgu", bufs=2, space="PSUM") as psgu,
            tc.tile_pool(name="psop", bufs=3, space="PSUM") as psop,
        ):
          for _rep in range(reps):
            blk0 = 0  # global block index
            for pi, sz in enumerate(piece_sizes):
                Tp = sz * TB
                h1 = h1p.tile([128, HB, Tp], bf16, tag="h1")
                # ---- phase 1: h1[h, t] = silu(G^T x) * (U^T x).
                # x blocks are separate tiles; each block's DMA is split in
                # two halves so the first matmuls start after ~half a block
                # has landed. gate/up weights stream once per piece
                # (hb-outer loop) from the scalar HWDGE queue.
                xws = []
                for tb in range(sz):
                    xw = xp.tile([128, KD, TB], bf16, tag="x")
                    # the startup-critical first block is quarter-split so
                    # the first matmuls start after ~512KB; later blocks use
                    # half-splits (fewer per-transfer overheads)
                    nsplit = 4 if (pi == 0 and tb == 0) else 2
                    q = KD // nsplit
                    for j in range(nsplit):
                        nc.sync.dma_start(
                            out=xw[:, j * q : (j + 1) * q, :],
                            in_=xb[
                                blk0 + tb, :, j * q * TB : (j + 1) * q * TB
                            ].rearrange("p (kd t) -> p kd t", t=TB),
                        )
                    xws.append(xw)
                # down-proj weights stream in dcb pairs (one transfer per 2
                # column tiles halves per-transfer overhead on the engines)
                paired = NCH % 2 == 0
                PD = 2 if paired else 1
                NPR = NCH // PD
                PFP = min(4, NPR)

                def dw_load(pr):
                    dw = dwp.tile([128, PD, HB, 128], bf16, tag="dw")
                    nc.scalar.dma_start(
                        out=dw,
                        in_=ds[pi][pr * PD : (pr + 1) * PD].rearrange(
                            "d p (hb c) -> p d hb c", c=128
                        ),
                    )
                    return dw

                dws = {}
                for hb in range(HB):
                    gw = wp.tile([128, KD, 128], bf16, tag="gw")
                    uw = wp.tile([128, KD, 128], bf16, tag="uw")
                    half = KD // 2
                    # startup-critical first weights are half-split; the
                    # rest load as single transfers
                    nsp = 2 if (pi == 0 and hb == 0) else 1
                    ws = KD // nsp
                    for w, src in ((gw, gs[pi]), (uw, us[pi])):
                        for j in range(nsp):
                            nc.scalar.dma_start(
                                out=w[:, j * ws : (j + 1) * ws, :],
                                in_=src[
                                    hb, :, j * ws * 128 : (j + 1) * ws * 128
                                ].rearrange("p (kd h) -> p kd h", h=128),
                            )
                    # hoisted down-proj prefetch: the first PFP dw pairs load
                    # before phase 2 so it begins with a deep resident
                    # pipeline. For the first piece defer to the last hb --
                    # during startup every HBM byte is needed for x/gate/up.
                    if hb == (0 if pi else HB - 1):
                        for pr in range(PFP):
                            dws[pr] = dw_load(pr)
                    for tb in range(sz):
                        xw = xws[tb]
                        psg = psgu.tile([128, TB], f32, tag="psg")
                        psu = psgu.tile([128, TB], f32, tag="psu")
                        for kd in range(KD):
                            nc.tensor.matmul(
                                psg,
                                gw[:, kd, :],
                                xw[:, kd, :],
                                start=(kd == 0),
                                stop=(kd == KD - 1),
                            )
                        for kd in range(KD):
                            nc.tensor.matmul(
                                psu,
                                uw[:, kd, :],
                                xw[:, kd, :],
                                start=(kd == 0),
                                stop=(kd == KD - 1),
                            )
                        act = actp.tile([128, TB], f32, tag="act")
                        nc.scalar.activation(
                            act, psg, mybir.ActivationFunctionType.Silu
                        )
                        nc.vector.tensor_mul(
                            h1[:, hb, tb * TB : (tb + 1) * TB], act, psu
                        )
                # ---- phase 2: yT[c, t] = sum_h D^T[h, c] * h1[h, t]
                # stationary = 128x128 D^T column tiles, moving = h1
                # (already resident). dw pair-prefetch rolls PFP pairs ahead
                # on the scalar queue (idle during phase 2). y writes are
                # batched: one transfer per dcb (sz>=2) or per dcb pair
                # (sz==1) so each moves >=2KB/partition.
                t_lo = blk0 * TB
                for pr in range(NPR):
                    if pr + PFP < NPR:
                        dws[pr + PFP] = dw_load(pr + PFP)
                    dw = dws.pop(pr)
                    if sz == 1 and paired:
                        otd = otp.tile([128, PD, 512], bf16, tag="ot2")
                    for s in range(PD):
                        dcb = pr * PD + s
                        if sz != 1 or not paired:
                            otd = otp.tile([128, Tp], bf16, tag="otd")
                        for tcol in range(Tp // 512):
                            pso = psop.tile([128, 512], f32, tag="pso")
                            for hb in range(HB):
                                nc.tensor.matmul(
                                    pso,
                                    dw[:, s, hb, :],
                                    h1[:, hb, tcol * 512 : (tcol + 1) * 512],
                                    start=(hb == 0),
                                    stop=(hb == HB - 1),
                                )
                            dst = (
                                otd[:, s, :]
                                if (sz == 1 and paired)
                                else otd[:, tcol * 512 : (tcol + 1) * 512]
                            )
                            nc.vector.tensor_copy(dst, pso)
                        if sz != 1 or not paired:
                            nc.sync.dma_start(
                                out=y[dcb, :, t_lo : t_lo + Tp], in_=otd
                            )
                    if sz == 1 and paired:
                        nc.sync.dma_start(
                            out=y[
                                pr * PD : (pr + 1) * PD, :, t_lo : t_lo + 512
                            ].rearrange("d p t -> p d t"),
                            in_=otd,
                        )
                blk0 += sz
    nc.move_matmul_waits_to_ldweights()
    nc.compile()
    _PROGRAM_CACHE[key] = nc
    return nc


# --------------------------------------------------------------------------
# host-side planning
# --------------------------------------------------------------------------
def plan_pieces(block_counts):
    """Decompose per-expert block counts into 8 cores x uniform piece sizes.

    Returns (piece_sizes, plans) where plans[core] = [(expert, block_start,
    nblocks), ...] with block_start in global padded block coordinates.
    Tries the (3, 1) split (balanced, min weight traffic); falls back to
    single-block pieces.
    """
    E = len(block_counts)
    starts = np.zeros(E, dtype=np.int64)
    np.cumsum(block_counts[:-1], out=starts[1:])
    total = int(np.sum(block_counts))

    if total == 4 * NCORES:
        # try k3[e] three-pieces + k1[e] singles with sum(k3) == 8
        k3 = [int(c) // 3 for c in block_counts]
        while sum(k3) > NCORES:
            e = max(range(E), key=lambda i: k3[i])
            k3[e] -= 1
        if sum(k3) == NCORES:
            threes, ones = [], []
            for e in range(E):
                b = int(block_counts[e])
                s = int(starts[e])
                for _ in range(k3[e]):
                    threes.append((e, s, 3))
                    s += 3
                while s < int(starts[e]) + b:
                    ones.append((e, s, 1))
                    s += 1
            assert len(threes) == NCORES and len(ones) == NCORES
            # pair same-expert pieces on the same core where possible;
            # SMALL piece first: its first matmuls need only ~2MB of x, so
            # the PE starts (and warms the clock) while the big piece's
            # x blocks prefetch; the big piece's robust phase 2 lands at
            # the tail, keeping the clock warm to the end
            plans = []
            used1 = [False] * NCORES
            for t in threes:
                j = next(
                    (
                        i
                        for i in range(NCORES)
                        if not used1[i] and ones[i][0] == t[0]
                    ),
                    None,
                )
                if j is None:
                    j = next(i for i in range(NCORES) if not used1[i])
                used1[j] = True
                plans.append([t, ones[j]])
            return (3, 1), plans

    # fallback: single-block pieces, padded to a multiple of NCORES with
    # dummy zero blocks (expert 0 weights, output discarded)
    per_core = -(-total // NCORES)
    pieces = []
    for e in range(E):
        for b in range(int(block_counts[e])):
            pieces.append((e, int(starts[e]) + b, 1))
    while len(pieces) < per_core * NCORES:
        pieces.append((0, -1, 1))  # dummy
    plans = [pieces[c * per_core : (c + 1) * per_core] for c in range(NCORES)]
    return tuple([1] * per_core), plans


def _pack_gu(w, KD, HB):
    """[hid, dim] f32 -> [HB, 128, KD*128] bf16 with
    out[hb, p, kd*128+h] = w[hb*128+h, kd*128+p]."""
    hid, dim = w.shape
    arr = w.reshape(HB, 128, KD, 128).transpose(0, 3, 2, 1)  # hb, p, kd, h
    return np.ascontiguousarray(arr.astype(BF16).reshape(HB, 128, KD * 128))


def _pack_d(w, NCH, HB):
    """[dim, hid] f32 -> [NCH, 128, HB*128] bf16 with
    out[cb, p, hb*128+c] = w[cb*128+c, hb*128+p]."""
    dim, hid = w.shape
    arr = w.reshape(NCH, 128, HB, 128).transpose(0, 3, 2, 1)  # cb, p, hb, c
    return np.ascontiguousarray(arr.astype(BF16).reshape(NCH, 128, HB * 128))


def _pack_x(xc, KD):
    """[T, dim] f32 -> [nblk, 128, KD*TB] bf16 with
    out[b, p, kd*TB+t] = x[b*TB+t, kd*128+p]."""
    T, dim = xc.shape
    nblk = T // TB
    arr = xc.reshape(nblk, TB, KD, 128).transpose(0, 3, 2, 1)  # b, p, kd, t
    return np.ascontiguousarray(arr.astype(BF16).reshape(nblk, 128, KD * TB))


def prepare(x, gate_proj, up_proj, down_proj, num_tokens_per_expert):
    """Host-side planning + per-core input construction.

    Returns (piece_sizes, plans, in_maps, scatter_info).
    """
    x = np.ascontiguousarray(np.asarray(x, dtype=np.float32))
    gate_proj = np.asarray(gate_proj, dtype=np.float32)
    up_proj = np.asarray(up_proj, dtype=np.float32)
    down_proj = np.asarray(down_proj, dtype=np.float32)
    counts = np.asarray(num_tokens_per_expert).astype(np.int64)

    T, dim = x.shape
    E, hid, _ = gate_proj.shape
    KD, HB, NCH = dim // 128, hid // 128, dim // 128

    # ---- pad each expert's token segment to a multiple of TB (no-op for the
    # staged problem where every count is already a multiple of 512)
    offs = np.concatenate([[0], np.cumsum(counts)])
    pad_counts = ((counts + TB - 1) // TB) * TB
    if np.array_equal(pad_counts, counts):
        x_pad = x
        pad_offs = offs
        padded = False
    else:
        pad_offs = np.concatenate([[0], np.cumsum(pad_counts)])
        x_pad = np.zeros((int(pad_offs[-1]), dim), dtype=np.float32)
        for e in range(E):
            x_pad[pad_offs[e] : pad_offs[e] + counts[e]] = x[offs[e] : offs[e + 1]]
        padded = True

    block_counts = pad_counts // TB
    piece_sizes, plans = plan_pieces(block_counts)

    # ---- per-expert packed bf16 weights (shared across cores)
    GP = [_pack_gu(gate_proj[e], KD, HB) for e in range(E)]
    UP = [_pack_gu(up_proj[e], KD, HB) for e in range(E)]
    DP = [_pack_d(down_proj[e], NCH, HB) for e in range(E)]

    in_maps = []
    for c in range(NCORES):
        plan = plans[c]
        xs = []
        for (e, bs, nb) in plan:
            if bs < 0:
                xs.append(np.zeros((nb * TB, dim), dtype=np.float32))
            else:
                xs.append(x_pad[bs * TB : (bs + nb) * TB])
        xc = np.concatenate(xs, axis=0) if len(xs) > 1 else xs[0]
        m = {"xb": _pack_x(xc, KD)}
        for i, (e, bs, nb) in enumerate(plan):
            m[f"g{i}"] = GP[e]
            m[f"u{i}"] = UP[e]
            m[f"d{i}"] = DP[e]
        in_maps.append(m)

    scatter_info = (T, dim, E, offs, pad_offs, counts, padded)
    return piece_sizes, plans, in_maps, scatter_info


def scatter(per_core_y, plans, scatter_info):
    """Assemble the full output from per-core y arrays."""
    T, dim, E, offs, pad_offs, counts, padded = scatter_info
    out_pad = np.empty((int(pad_offs[-1]), dim), dtype=np.float32)
    for c in range(NCORES):
        yc = np.asarray(per_core_y[c]).astype(np.float32)  # [NCH, 128, T_core]
        yc = yc.transpose(2, 0, 1).reshape(yc.shape[2], dim)  # [T_core, dim]
        t = 0
        for (e, bs, nb) in plans[c]:
            if bs >= 0:
                out_pad[bs * TB : (bs + nb) * TB] = yc[t : t + nb * TB]
            t += nb * TB
    if not padded:
        return out_pad
    out = np.empty((T, dim), dtype=np.float32)
    for e in range(E):
        out[offs[e] : offs[e + 1]] = out_pad[pad_offs[e] : pad_offs[e] + counts[e]]
    return out


def kernel(x, gate_proj, up_proj, down_proj, num_tokens_per_expert):
    piece_sizes, plans, in_maps, scatter_info = prepare(
        x, gate_proj, up_proj, down_proj, num_tokens_per_expert
    )
    dim = scatter_info[1]
    hid = np.asarray(gate_proj).shape[1]
    nc = build_program(piece_sizes, dim, hid)
    res = run_bass_kernel_spmd(nc, in_maps, core_ids=list(range(NCORES)))
    return scatter([res.results[c]["y"] for c in range(NCORES)], plans, scatter_info)
